# revision 23
# baseline (speedup 1.0000x reference)
# Trainium2 Bass kernel for pointer-generator coverage attention
# (nn_Attention_4191888080922).
#
# Math (per batch b):
#   dec   = s_t_hat[b] @ W_dp.T + b_dp                         [n]
#   att   = enc_feature[b] + dec[None,:] + cov[b][:,None]*W_c  [t, n]
#   e     = tanh(att)
#   score = e @ v_w  (+ v_b, dropped: softmax is shift-invariant)
#   attn  = softmax(score) * mask / sum(softmax(score) * mask)
#   c_t   = attn @ enc_outputs[b]
#   cov'  = cov + attn
#
# Distribution: data-parallel over batch. Core i owns batches [8i, 8i+8).
# No cross-core communication.
#
# t-axis permutation: the 2048 encoder positions of a batch are processed as
# 2 supertiles (sc) x 128 partitions (p) x 8 subtiles (c) with
#   t = sc*1024 + p*8 + c
# so every DMA is partition-major contiguous (16KB per partition for the big
# streams). Softmax is permutation-invariant; coverage/mask inputs and the
# attn/coverage_new outputs are (un)permuted host-side (layout only).
#
# Per-core dataflow (natural [t_partition, n_free] layout):
#   - 2MB DMA supertiles of enc_feature -> SBUF
#   - PE: PSUM = I.T @ enc_feat_chunk  (stream-copy into PSUM)
#             += [ones; cov_chunk].T @ [dec; W_c]  (K=2 rank-2 broadcast)
#   - ACT: e = tanh(PSUM) -> SBUF
#   - DVE: scores = reduce_sum(e * vw_broadcast) over n, grouped 4 subtiles
#     per instruction (tensor_tensor_reduce would fuse these but crashes the
#     exec unit under this runtime, so stock ops only)
#   - softmax on [128, 16] scores + PE 1-col matmuls for the
#     cross-partition sum and the 1/total broadcast
#   - PE: c_t = sum_chunks attn_col.T @ enc_out_chunk  (PSUM accumulate)

import os
from contextlib import ExitStack

import numpy as np

import concourse.bass as bass
import concourse.bacc as bacc
import concourse.tile as tile
from concourse import mybir
from concourse._compat import with_exitstack
from concourse.bass_utils import run_bass_kernel_spmd

F32 = mybir.dt.float32
BF16 = mybir.dt.bfloat16

B, TK, H2 = 64, 2048, 512
NCORES = 8
BPC = B // NCORES          # batches per core
P = 128                    # partitions
NT = TK // P               # 16 t-chunks of 128 per batch
SUP = 8                    # t-chunks per DMA supertile (2MB)
NSUP = NT // SUP           # supertiles per batch
KC = H2 // P               # 4 contraction chunks for the dec matmul
SUBG = 4                   # subtiles grouped per DVE mul/reduce instruction
# Matmul-operand dtype for the big streamed paths. fp32 matmuls run at 4
# cycles/row; float32r (4-byte, reduced-mantissa) and bf16 run at 1. float32r
# must be materialized by the producer (SWDGE cast-DMA / engine write).
MM_MODE = "r32"            # "f32" | "r32" | "bf16"
EDT = {"f32": F32, "r32": mybir.dt.float32r, "bf16": BF16}[MM_MODE]
E_CAST = MM_MODE != "f32"
# dtype of the DVE score path (bf16 doubles tensor_tensor throughput)
DDT = BF16 if MM_MODE == "bf16" else F32


@with_exitstack
def _body(ctx: ExitStack, tc, io):
    nc = tc.nc
    ef, eo, ocp, maskp, covp, sTp, wTp, bdp, wc, vw, ident = (
        io["ef"], io["eo"], io["ocp"], io["maskp"], io["covp"], io["sTp"],
        io["wTp"], io["bdp"], io["wc"], io["vw"], io["ident"],
    )
    ct_o, attn_o, cvn_o = io["c_t"], io["attn_p"], io["cvn_p"]

    const = ctx.enter_context(tc.tile_pool(name="const", bufs=1))
    dram = ctx.enter_context(tc.tile_pool(name="dram", bufs=1, space="DRAM"))
    efp = ctx.enter_context(tc.tile_pool(name="efp", bufs=2))
    eop = ctx.enter_context(tc.tile_pool(name="eop", bufs=2))
    ep = ctx.enter_context(tc.tile_pool(name="ep", bufs=3))
    scrp = ctx.enter_context(tc.tile_pool(name="scrp", bufs=2))
    smallp = ctx.enter_context(tc.tile_pool(name="smallp", bufs=2))
    scorep = ctx.enter_context(tc.tile_pool(name="scorep", bufs=2))
    lhsp = ctx.enter_context(tc.tile_pool(name="lhsp", bufs=2))
    psA = ctx.enter_context(tc.tile_pool(name="psA", bufs=3, space="PSUM"))
    psC = ctx.enter_context(tc.tile_pool(name="psC", bufs=2, space="PSUM"))
    psS = ctx.enter_context(tc.tile_pool(name="psS", bufs=2, space="PSUM"))

    # --- constants ---
    ident_sb = const.tile([P, P], EDT)
    if E_CAST:
        nc.gpsimd.dma_start(out=ident_sb, in_=ident[:, :])   # SWDGE casts
    else:
        nc.sync.dma_start(out=ident_sb, in_=ident[:, :])

    ones_row = const.tile([1, P], F32)
    nc.vector.memset(ones_row, 1.0)
    ones_col = const.tile([P, 1], F32)
    nc.vector.memset(ones_col, 1.0)

    vw_sb = const.tile([1, H2], F32)
    nc.sync.dma_start(out=vw_sb, in_=vw[:, :])
    bdp_sb = const.tile([1, H2], F32)
    nc.sync.dma_start(out=bdp_sb, in_=bdp[:, :])

    maskp_sb = const.tile([P, BPC, NT], F32)
    nc.sync.dma_start(out=maskp_sb, in_=maskp.rearrange("p (b c) -> p b c", b=BPC))
    covp_sb = const.tile([P, BPC, NT], F32)
    nc.sync.dma_start(out=covp_sb, in_=covp.rearrange("p (b c) -> p b c", b=BPC))

    # vw broadcast to all 128 partitions (and replicated SUBG times along the
    # free axis) via K=1 ones matmuls
    vw_bc = const.tile([P, SUBG, H2], DDT)
    for g in range(SUBG):
        vbc_ps = psA.tile([P, H2], F32, tag="att")
        nc.tensor.matmul(vbc_ps, ones_row, vw_sb, start=True, stop=True)
        nc.scalar.copy(out=vw_bc[:, g, :], in_=vbc_ps)

    # --- dec = s_t_hat @ W_dp.T + b_dp, all BPC batches at once ---
    sT_sb = const.tile([P, KC, BPC], F32)
    nc.sync.dma_start(out=sT_sb, in_=sTp.rearrange("p (kc b) -> p kc b", kc=KC))
    wT_sb = const.tile([P, KC, H2], F32)
    nc.sync.dma_start(out=wT_sb, in_=wTp.rearrange("p (kc j) -> p kc j", kc=KC))

    dec_ps = psC.tile([BPC, H2], F32, tag="ct")
    for kc in range(KC):
        nc.tensor.matmul(dec_ps, sT_sb[:, kc, :], wT_sb[:, kc, :],
                         start=(kc == 0), stop=False)
    nc.tensor.matmul(dec_ps, ones_row[0:1, 0:BPC], bdp_sb, start=False, stop=True)
    dec_sb = const.tile([BPC, H2], F32)
    nc.scalar.copy(out=dec_sb, in_=dec_ps)
    # Round-trip through DRAM so each batch's dec row can be reloaded at
    # partition 0 (engines cannot move data across partitions).
    dec_dram = dram.tile([BPC, H2], F32)
    nc.sync.dma_start(out=dec_dram, in_=dec_sb)

    for b in range(BPC):
        # per-batch broadcast operands (SWDGE casts to EDT)
        oc_sb = lhsp.tile([2, TK], EDT, tag="oc")   # row0=ones, row1=perm cov[b]
        nc.gpsimd.dma_start(out=oc_sb, in_=ocp[b])
        rhs_dw = lhsp.tile([2, H2], EDT, tag="dw")  # row0=dec[b], row1=W_c
        nc.gpsimd.dma_start(out=rhs_dw[0:1, :], in_=dec_dram[b:b + 1, :])
        nc.gpsimd.dma_start(out=rhs_dw[1:2, :], in_=wc[:, :])

        scores = scorep.tile([P, NT], F32)
        for sc in range(NSUP):
            eft = efp.tile([P, SUP, H2], EDT)
            src = (ef[b, sc * SUP * P:(sc + 1) * SUP * P, :]
                   .rearrange("(p c) n -> p c n", p=P))
            if E_CAST:
                nc.gpsimd.dma_start(out=eft, in_=src)
            else:
                nc.sync.dma_start(out=eft, in_=src)
            for g in range(SUP // SUBG):
                e_big = ep.tile([P, SUBG, H2], DDT)
                for cc in range(SUBG):
                    c = g * SUBG + cc
                    col = sc * SUP + c
                    att_ps = psA.tile([P, H2], F32, tag="att")
                    nc.tensor.matmul(att_ps, ident_sb, eft[:, c, :],
                                     start=True, stop=False)
                    nc.tensor.matmul(att_ps,
                                     oc_sb[:, col * P:(col + 1) * P],
                                     rhs_dw, start=False, stop=True)
                    nc.scalar.activation(out=e_big[:, cc, :], in_=att_ps,
                                         func=mybir.ActivationFunctionType.Tanh)
                scr = scrp.tile([P, SUBG, H2], DDT)
                nc.vector.tensor_mul(scr, e_big, vw_bc)
                col0 = sc * SUP + g * SUBG
                nc.vector.reduce_sum(out=scores[:, col0:col0 + SUBG], in_=scr,
                                     axis=mybir.AxisListType.X)

        # --- softmax over all 2048 t (v_b and max-shift cancel) ---
        expm = smallp.tile([P, NT], F32, tag="expm")
        nc.scalar.activation(out=expm, in_=scores,
                             func=mybir.ActivationFunctionType.Exp)
        masked = smallp.tile([P, NT], F32, tag="masked")
        nc.vector.tensor_mul(masked, expm, maskp_sb[:, b, :])
        colsum = smallp.tile([P, 1], F32, tag="colsum")
        nc.vector.reduce_sum(out=colsum, in_=masked, axis=mybir.AxisListType.X)
        tot_ps = psS.tile([1, 1], F32, tag="smallps")
        nc.tensor.matmul(tot_ps, colsum, ones_col, start=True, stop=True)
        recip = smallp.tile([1, 1], F32, tag="recip")
        nc.vector.reciprocal(out=recip, in_=tot_ps)
        rb_ps = psS.tile([P, 1], F32, tag="smallps")
        nc.tensor.matmul(rb_ps, ones_row, recip, start=True, stop=True)
        rb_sb = smallp.tile([P, 1], F32, tag="rbsb")
        nc.scalar.copy(out=rb_sb, in_=rb_ps)
        attn_t = smallp.tile([P, NT], F32, tag="attn")
        nc.vector.tensor_scalar_mul(attn_t, masked, rb_sb)

        covnew = smallp.tile([P, NT], F32, tag="covnew")
        nc.vector.tensor_add(covnew, covp_sb[:, b, :], attn_t)
        nc.gpsimd.dma_start(out=attn_o[b], in_=attn_t)
        nc.gpsimd.dma_start(out=cvn_o[b], in_=covnew)

        # --- c_t = attn @ enc_outputs[b] ---
        if E_CAST:
            attn_mm = smallp.tile([P, NT], EDT, tag="attnbf")
            nc.vector.tensor_copy(attn_mm, attn_t)
        else:
            attn_mm = attn_t
        ct_ps = psC.tile([1, H2], F32, tag="ct")
        for sc in range(NSUP):
            eot = eop.tile([P, SUP, H2], EDT)
            src = (eo[b, sc * SUP * P:(sc + 1) * SUP * P, :]
                   .rearrange("(p c) n -> p c n", p=P))
            if E_CAST:
                nc.gpsimd.dma_start(out=eot, in_=src)
            else:
                nc.sync.dma_start(out=eot, in_=src)
            for c in range(SUP):
                col = sc * SUP + c
                nc.tensor.matmul(ct_ps, attn_mm[:, col:col + 1],
                                 eot[:, c, :],
                                 start=(col == 0), stop=(col == NT - 1))
        ct_sb = smallp.tile([1, H2], F32, tag="ctsb")
        nc.scalar.copy(out=ct_sb, in_=ct_ps)
        nc.gpsimd.dma_start(out=ct_o[b:b + 1, :], in_=ct_sb)


def _build_nc() -> bass.Bass:
    nc = bacc.Bacc(None, target_bir_lowering=False)
    io = {}
    io["ef"] = nc.dram_tensor("ef", (BPC, TK, H2), F32, kind="ExternalInput")
    io["eo"] = nc.dram_tensor("eo", (BPC, TK, H2), F32, kind="ExternalInput")
    io["ocp"] = nc.dram_tensor("ocp", (BPC, 2, TK), F32, kind="ExternalInput")
    io["maskp"] = nc.dram_tensor("maskp", (P, BPC * NT), F32, kind="ExternalInput")
    io["covp"] = nc.dram_tensor("covp", (P, BPC * NT), F32, kind="ExternalInput")
    io["sTp"] = nc.dram_tensor("sTp", (P, KC * BPC), F32, kind="ExternalInput")
    io["wTp"] = nc.dram_tensor("wTp", (P, KC * H2), F32, kind="ExternalInput")
    io["bdp"] = nc.dram_tensor("bdp", (1, H2), F32, kind="ExternalInput")
    io["wc"] = nc.dram_tensor("wc", (1, H2), F32, kind="ExternalInput")
    io["vw"] = nc.dram_tensor("vw", (1, H2), F32, kind="ExternalInput")
    io["ident"] = nc.dram_tensor("ident", (P, P), F32, kind="ExternalInput")
    io["c_t"] = nc.dram_tensor("c_t", (BPC, H2), F32, kind="ExternalOutput")
    io["attn_p"] = nc.dram_tensor("attn_p", (BPC, P, NT), F32,
                                  kind="ExternalOutput")
    io["cvn_p"] = nc.dram_tensor("cvn_p", (BPC, P, NT), F32,
                                 kind="ExternalOutput")
    with tile.TileContext(nc) as tc:
        _body(tc, io)
    nc.compile()
    return nc


def _perm_bt(x):
    """[BPC, 2048] batch-shard -> permuted [P, BPC*NT]: out[p, b*16+sc*8+c] =
    x[b, sc*1024 + p*8 + c]."""
    # x -> [b, sc, p, c] -> [p, b, sc, c]
    return np.ascontiguousarray(
        x.reshape(BPC, NSUP, P, SUP).transpose(2, 0, 1, 3).reshape(P, BPC * NT)
    )


def _unperm_bt(y):
    """Inverse of the per-batch t-permutation for outputs shaped [BPC, P, NT]."""
    # y[b, p, sc*8+c] -> x[b, sc*1024 + p*8 + c]
    return np.ascontiguousarray(
        y.reshape(BPC, P, NSUP, SUP).transpose(0, 2, 1, 3).reshape(BPC, TK)
    )


def _make_in_map(i, s_t_hat, encoder_outputs, encoder_feature,
                 enc_padding_mask, coverage, wT, bdp, wc, vw, ident):
    sl = slice(i * BPC, (i + 1) * BPC)
    cov = coverage[sl]
    ocp = np.empty((BPC, 2, TK), dtype=np.float32)
    ocp[:, 0, :] = 1.0
    # lhsT layout: row1 slice [col*128:(col+1)*128] must hold
    # cov[b, sc*1024 + p*8 + c] for p=0..127 at col=sc*8+c
    ocp[:, 1, :] = (cov.reshape(BPC, NSUP, P, SUP).transpose(0, 1, 3, 2)
                    .reshape(BPC, TK))
    sT = s_t_hat[sl].T  # [H2, BPC]
    sTp = np.ascontiguousarray(
        sT.reshape(KC, P, BPC).transpose(1, 0, 2).reshape(P, KC * BPC))
    return {
        "ef": encoder_feature[sl],
        "eo": encoder_outputs[sl],
        "ocp": ocp,
        "maskp": _perm_bt(enc_padding_mask[sl]),
        "covp": _perm_bt(cov),
        "sTp": sTp,
        "wTp": wT,
        "bdp": bdp,
        "wc": wc,
        "vw": vw,
        "ident": ident,
    }


_NC_CACHE = None
last_perf = {}


def kernel(s_t_hat, encoder_outputs, encoder_feature, enc_padding_mask, coverage,
           W_dp, b_dp, W_c, v_w, v_b):
    global _NC_CACHE
    if _NC_CACHE is None:
        _NC_CACHE = _build_nc()
    nc = _NC_CACHE

    s_t_hat = np.asarray(s_t_hat, dtype=np.float32)
    encoder_outputs = np.ascontiguousarray(encoder_outputs, dtype=np.float32)
    encoder_feature = np.ascontiguousarray(encoder_feature, dtype=np.float32)
    enc_padding_mask = np.ascontiguousarray(enc_padding_mask, dtype=np.float32)
    coverage = np.asarray(coverage, dtype=np.float32)
    # W_dp.T laid out partition-major: wTp[p, kc*H2+j] = W_dp[j, kc*128+p]
    wT = np.ascontiguousarray(
        np.asarray(W_dp, dtype=np.float32).T
        .reshape(KC, P, H2).transpose(1, 0, 2).reshape(P, KC * H2))
    bdp = np.asarray(b_dp, dtype=np.float32).reshape(1, H2)
    wc = np.asarray(W_c, dtype=np.float32).reshape(1, H2)
    vw = np.asarray(v_w, dtype=np.float32).reshape(1, H2)
    ident = np.eye(P, dtype=np.float32)

    in_maps = [
        _make_in_map(i, s_t_hat, encoder_outputs, encoder_feature,
                     enc_padding_mask, coverage, wT, bdp, wc, vw, ident)
        for i in range(NCORES)
    ]

    trace = os.environ.get("TRN_KERNEL_TRACE", "0") == "1"
    res = run_bass_kernel_spmd(nc, in_maps, core_ids=list(range(NCORES)),
                               trace=trace)
    last_perf["exec_time_ns"] = res.exec_time_ns
    last_perf["profile_json"] = res.profile_json
    outs = res.results

    c_t = np.concatenate([outs[i]["c_t"] for i in range(NCORES)], axis=0)
    attn = np.concatenate(
        [_unperm_bt(outs[i]["attn_p"]) for i in range(NCORES)], axis=0)
    cvn = np.concatenate(
        [_unperm_bt(outs[i]["cvn_p"]) for i in range(NCORES)], axis=0)
    return c_t, attn, cvn


# revision 25
# speedup vs baseline: 1.2532x; 1.2532x over previous
# Trainium2 Bass kernel for pointer-generator coverage attention
# (nn_Attention_4191888080922).
#
# Math (per batch b):
#   dec   = s_t_hat[b] @ W_dp.T + b_dp                         [n]
#   att   = enc_feature[b] + dec[None,:] + cov[b][:,None]*W_c  [t, n]
#   e     = tanh(att)
#   score = e @ v_w  (+ v_b, dropped: softmax is shift-invariant)
#   attn  = softmax(score) * mask / sum(softmax(score) * mask)
#   c_t   = attn @ enc_outputs[b]
#   cov'  = cov + attn
#
# Distribution: data-parallel over batch. Core i owns batches [8i, 8i+8).
# No cross-core communication.
#
# t-axis permutation: the 2048 encoder positions of a batch are processed as
# 2 supertiles (sc) x 128 partitions (p) x 8 subtiles (c) with
#   t = sc*1024 + p*8 + c
# so every DMA is partition-major contiguous (16KB per partition for the big
# streams). Softmax is permutation-invariant; coverage/mask inputs and the
# attn/coverage_new outputs are (un)permuted host-side (layout only).
#
# Per-core dataflow (natural [t_partition, n_free] layout):
#   - 2MB DMA supertiles of enc_feature -> SBUF
#   - PE: PSUM = I.T @ enc_feat_chunk  (stream-copy into PSUM)
#             += [ones; cov_chunk].T @ [dec; W_c]  (K=2 rank-2 broadcast)
#   - ACT: e = tanh(PSUM) -> SBUF
#   - DVE: scores = reduce_sum(e * vw_broadcast) over n, grouped 4 subtiles
#     per instruction (tensor_tensor_reduce would fuse these but crashes the
#     exec unit under this runtime, so stock ops only)
#   - softmax on [128, 16] scores + PE 1-col matmuls for the
#     cross-partition sum and the 1/total broadcast
#   - PE: c_t = sum_chunks attn_col.T @ enc_out_chunk  (PSUM accumulate)

import os
from contextlib import ExitStack

import numpy as np

import concourse.bass as bass
import concourse.bacc as bacc
import concourse.tile as tile
from concourse import mybir
from concourse._compat import with_exitstack
from concourse.bass_utils import run_bass_kernel_spmd

F32 = mybir.dt.float32
BF16 = mybir.dt.bfloat16

B, TK, H2 = 64, 2048, 512
NCORES = 8
BPC = B // NCORES          # batches per core
P = 128                    # partitions
NT = TK // P               # 16 t-chunks of 128 per batch
SUP = 8                    # t-chunks per DMA supertile (2MB)
NSUP = NT // SUP           # supertiles per batch
KC = H2 // P               # 4 contraction chunks for the dec matmul
SUBG = 4                   # subtiles grouped per DVE mul/reduce instruction
# Matmul-operand dtype for the big streamed paths. fp32 matmuls run at 4
# cycles/row; float32r (4-byte, reduced-mantissa) and bf16 run at 1. float32r
# must be materialized by the producer (SWDGE cast-DMA / engine write).
MM_MODE = "r32"            # "f32" | "r32" | "bf16"
EDT = {"f32": F32, "r32": mybir.dt.float32r, "bf16": BF16}[MM_MODE]
E_CAST = MM_MODE != "f32"
# dtype of the DVE score path (bf16 doubles tensor_tensor throughput)
DDT = BF16 if MM_MODE == "bf16" else F32
# benchmarking aid: repeat the whole per-core computation REPS times so the
# per-rep device time can be extracted from wall-clock differences
REPS = int(os.environ.get("TRN_KERNEL_REPS", "1"))


@with_exitstack
def _body(ctx: ExitStack, tc, io):
    nc = tc.nc
    ef, eo, ocp, maskp, covp, sTp, wTp, bdp, wc, vw, ident = (
        io["ef"], io["eo"], io["ocp"], io["maskp"], io["covp"], io["sTp"],
        io["wTp"], io["bdp"], io["wc"], io["vw"], io["ident"],
    )
    ct_o, attn_o, cvn_o = io["c_t"], io["attn_p"], io["cvn_p"]

    const = ctx.enter_context(tc.tile_pool(name="const", bufs=1))
    dram = ctx.enter_context(tc.tile_pool(name="dram", bufs=1, space="DRAM"))
    efp = ctx.enter_context(tc.tile_pool(name="efp", bufs=2))
    eop = ctx.enter_context(tc.tile_pool(name="eop", bufs=2))
    ep = ctx.enter_context(tc.tile_pool(name="ep", bufs=3))
    scrp = ctx.enter_context(tc.tile_pool(name="scrp", bufs=2))
    smallp = ctx.enter_context(tc.tile_pool(name="smallp", bufs=2))
    scorep = ctx.enter_context(tc.tile_pool(name="scorep", bufs=2))
    lhsp = ctx.enter_context(tc.tile_pool(name="lhsp", bufs=2))
    psA = ctx.enter_context(tc.tile_pool(name="psA", bufs=3, space="PSUM"))
    psC = ctx.enter_context(tc.tile_pool(name="psC", bufs=2, space="PSUM"))
    psS = ctx.enter_context(tc.tile_pool(name="psS", bufs=2, space="PSUM"))

    # --- constants ---
    ident_sb = const.tile([P, P], EDT)
    if E_CAST:
        nc.gpsimd.dma_start(out=ident_sb, in_=ident[:, :])   # SWDGE casts
    else:
        nc.sync.dma_start(out=ident_sb, in_=ident[:, :])

    ones_row = const.tile([1, P], F32)
    nc.vector.memset(ones_row, 1.0)
    ones_col = const.tile([P, 1], F32)
    nc.vector.memset(ones_col, 1.0)

    vw_sb = const.tile([1, H2], F32)
    nc.sync.dma_start(out=vw_sb, in_=vw[:, :])
    bdp_sb = const.tile([1, H2], F32)
    nc.sync.dma_start(out=bdp_sb, in_=bdp[:, :])

    maskp_sb = const.tile([P, BPC, NT], F32)
    nc.sync.dma_start(out=maskp_sb, in_=maskp.rearrange("p (b c) -> p b c", b=BPC))
    covp_sb = const.tile([P, BPC, NT], F32)
    nc.sync.dma_start(out=covp_sb, in_=covp.rearrange("p (b c) -> p b c", b=BPC))

    # vw broadcast to all 128 partitions (and replicated SUBG times along the
    # free axis) via K=1 ones matmuls
    vw_bc = const.tile([P, SUBG, H2], DDT)
    for g in range(SUBG):
        vbc_ps = psA.tile([P, H2], F32, tag="att")
        nc.tensor.matmul(vbc_ps, ones_row, vw_sb, start=True, stop=True)
        nc.scalar.copy(out=vw_bc[:, g, :], in_=vbc_ps)

    # --- dec = s_t_hat @ W_dp.T + b_dp, all BPC batches at once ---
    sT_sb = const.tile([P, KC, BPC], F32)
    nc.sync.dma_start(out=sT_sb, in_=sTp.rearrange("p (kc b) -> p kc b", kc=KC))
    wT_sb = const.tile([P, KC, H2], F32)
    nc.sync.dma_start(out=wT_sb, in_=wTp.rearrange("p (kc j) -> p kc j", kc=KC))

    dec_ps = psC.tile([BPC, H2], F32, tag="ct")
    for kc in range(KC):
        nc.tensor.matmul(dec_ps, sT_sb[:, kc, :], wT_sb[:, kc, :],
                         start=(kc == 0), stop=False)
    nc.tensor.matmul(dec_ps, ones_row[0:1, 0:BPC], bdp_sb, start=False, stop=True)
    dec_sb = const.tile([BPC, H2], F32)
    nc.scalar.copy(out=dec_sb, in_=dec_ps)
    # Round-trip through DRAM so each batch's dec row can be reloaded at
    # partition 0 (engines cannot move data across partitions).
    dec_dram = dram.tile([BPC, H2], F32)
    nc.sync.dma_start(out=dec_dram, in_=dec_sb)

    for b in [b for _ in range(REPS) for b in range(BPC)]:
        # per-batch broadcast operands (SWDGE casts to EDT)
        oc_sb = lhsp.tile([2, TK], EDT, tag="oc")   # row0=ones, row1=perm cov[b]
        nc.gpsimd.dma_start(out=oc_sb, in_=ocp[b])
        rhs_dw = lhsp.tile([2, H2], EDT, tag="dw")  # row0=dec[b], row1=W_c
        nc.gpsimd.dma_start(out=rhs_dw[0:1, :], in_=dec_dram[b:b + 1, :])
        nc.gpsimd.dma_start(out=rhs_dw[1:2, :], in_=wc[:, :])

        scores = scorep.tile([P, NT], F32)
        for sc in range(NSUP):
            eft = efp.tile([P, SUP, H2], EDT)
            src = (ef[b, sc * SUP * P:(sc + 1) * SUP * P, :]
                   .rearrange("(p c) n -> p c n", p=P))
            if E_CAST:
                nc.gpsimd.dma_start(out=eft, in_=src)
            else:
                nc.sync.dma_start(out=eft, in_=src)
            for g in range(SUP // SUBG):
                e_big = ep.tile([P, SUBG, H2], DDT)
                for cc in range(SUBG):
                    c = g * SUBG + cc
                    col = sc * SUP + c
                    att_ps = psA.tile([P, H2], F32, tag="att")
                    nc.tensor.matmul(att_ps, ident_sb, eft[:, c, :],
                                     start=True, stop=False)
                    nc.tensor.matmul(att_ps,
                                     oc_sb[:, col * P:(col + 1) * P],
                                     rhs_dw, start=False, stop=True)
                    nc.scalar.activation(out=e_big[:, cc, :], in_=att_ps,
                                         func=mybir.ActivationFunctionType.Tanh)
                scr = scrp.tile([P, SUBG, H2], DDT)
                nc.vector.tensor_mul(scr, e_big, vw_bc)
                col0 = sc * SUP + g * SUBG
                nc.vector.reduce_sum(out=scores[:, col0:col0 + SUBG], in_=scr,
                                     axis=mybir.AxisListType.X)

        # --- softmax over all 2048 t (v_b and max-shift cancel) ---
        expm = smallp.tile([P, NT], F32, tag="expm")
        nc.scalar.activation(out=expm, in_=scores,
                             func=mybir.ActivationFunctionType.Exp)
        masked = smallp.tile([P, NT], F32, tag="masked")
        nc.vector.tensor_mul(masked, expm, maskp_sb[:, b, :])
        colsum = smallp.tile([P, 1], F32, tag="colsum")
        nc.vector.reduce_sum(out=colsum, in_=masked, axis=mybir.AxisListType.X)
        tot_ps = psS.tile([1, 1], F32, tag="smallps")
        nc.tensor.matmul(tot_ps, colsum, ones_col, start=True, stop=True)
        recip = smallp.tile([1, 1], F32, tag="recip")
        nc.vector.reciprocal(out=recip, in_=tot_ps)
        rb_ps = psS.tile([P, 1], F32, tag="smallps")
        nc.tensor.matmul(rb_ps, ones_row, recip, start=True, stop=True)
        rb_sb = smallp.tile([P, 1], F32, tag="rbsb")
        nc.scalar.copy(out=rb_sb, in_=rb_ps)
        attn_t = smallp.tile([P, NT], F32, tag="attn")
        nc.vector.tensor_scalar_mul(attn_t, masked, rb_sb)

        covnew = smallp.tile([P, NT], F32, tag="covnew")
        nc.vector.tensor_add(covnew, covp_sb[:, b, :], attn_t)
        nc.gpsimd.dma_start(out=attn_o[b], in_=attn_t)
        nc.gpsimd.dma_start(out=cvn_o[b], in_=covnew)

        # --- c_t = attn @ enc_outputs[b] ---
        if E_CAST:
            attn_mm = smallp.tile([P, NT], EDT, tag="attnbf")
            nc.vector.tensor_copy(attn_mm, attn_t)
        else:
            attn_mm = attn_t
        ct_ps = psC.tile([1, H2], F32, tag="ct")
        for sc in range(NSUP):
            eot = eop.tile([P, SUP, H2], EDT)
            src = (eo[b, sc * SUP * P:(sc + 1) * SUP * P, :]
                   .rearrange("(p c) n -> p c n", p=P))
            if E_CAST:
                nc.gpsimd.dma_start(out=eot, in_=src)
            else:
                nc.sync.dma_start(out=eot, in_=src)
            for c in range(SUP):
                col = sc * SUP + c
                nc.tensor.matmul(ct_ps, attn_mm[:, col:col + 1],
                                 eot[:, c, :],
                                 start=(col == 0), stop=(col == NT - 1))
        ct_sb = smallp.tile([1, H2], F32, tag="ctsb")
        nc.scalar.copy(out=ct_sb, in_=ct_ps)
        nc.gpsimd.dma_start(out=ct_o[b:b + 1, :], in_=ct_sb)


def _build_nc() -> bass.Bass:
    nc = bacc.Bacc(None, target_bir_lowering=False)
    io = {}
    io["ef"] = nc.dram_tensor("ef", (BPC, TK, H2), F32, kind="ExternalInput")
    io["eo"] = nc.dram_tensor("eo", (BPC, TK, H2), F32, kind="ExternalInput")
    io["ocp"] = nc.dram_tensor("ocp", (BPC, 2, TK), F32, kind="ExternalInput")
    io["maskp"] = nc.dram_tensor("maskp", (P, BPC * NT), F32, kind="ExternalInput")
    io["covp"] = nc.dram_tensor("covp", (P, BPC * NT), F32, kind="ExternalInput")
    io["sTp"] = nc.dram_tensor("sTp", (P, KC * BPC), F32, kind="ExternalInput")
    io["wTp"] = nc.dram_tensor("wTp", (P, KC * H2), F32, kind="ExternalInput")
    io["bdp"] = nc.dram_tensor("bdp", (1, H2), F32, kind="ExternalInput")
    io["wc"] = nc.dram_tensor("wc", (1, H2), F32, kind="ExternalInput")
    io["vw"] = nc.dram_tensor("vw", (1, H2), F32, kind="ExternalInput")
    io["ident"] = nc.dram_tensor("ident", (P, P), F32, kind="ExternalInput")
    io["c_t"] = nc.dram_tensor("c_t", (BPC, H2), F32, kind="ExternalOutput")
    io["attn_p"] = nc.dram_tensor("attn_p", (BPC, P, NT), F32,
                                  kind="ExternalOutput")
    io["cvn_p"] = nc.dram_tensor("cvn_p", (BPC, P, NT), F32,
                                 kind="ExternalOutput")
    with tile.TileContext(nc) as tc:
        _body(tc, io)
    nc.compile()
    return nc


def _perm_bt(x):
    """[BPC, 2048] batch-shard -> permuted [P, BPC*NT]: out[p, b*16+sc*8+c] =
    x[b, sc*1024 + p*8 + c]."""
    # x -> [b, sc, p, c] -> [p, b, sc, c]
    return np.ascontiguousarray(
        x.reshape(BPC, NSUP, P, SUP).transpose(2, 0, 1, 3).reshape(P, BPC * NT)
    )


def _unperm_bt(y):
    """Inverse of the per-batch t-permutation for outputs shaped [BPC, P, NT]."""
    # y[b, p, sc*8+c] -> x[b, sc*1024 + p*8 + c]
    return np.ascontiguousarray(
        y.reshape(BPC, P, NSUP, SUP).transpose(0, 2, 1, 3).reshape(BPC, TK)
    )


def _make_in_map(i, s_t_hat, encoder_outputs, encoder_feature,
                 enc_padding_mask, coverage, wT, bdp, wc, vw, ident):
    sl = slice(i * BPC, (i + 1) * BPC)
    cov = coverage[sl]
    ocp = np.empty((BPC, 2, TK), dtype=np.float32)
    ocp[:, 0, :] = 1.0
    # lhsT layout: row1 slice [col*128:(col+1)*128] must hold
    # cov[b, sc*1024 + p*8 + c] for p=0..127 at col=sc*8+c
    ocp[:, 1, :] = (cov.reshape(BPC, NSUP, P, SUP).transpose(0, 1, 3, 2)
                    .reshape(BPC, TK))
    sT = s_t_hat[sl].T  # [H2, BPC]
    sTp = np.ascontiguousarray(
        sT.reshape(KC, P, BPC).transpose(1, 0, 2).reshape(P, KC * BPC))
    return {
        "ef": encoder_feature[sl],
        "eo": encoder_outputs[sl],
        "ocp": ocp,
        "maskp": _perm_bt(enc_padding_mask[sl]),
        "covp": _perm_bt(cov),
        "sTp": sTp,
        "wTp": wT,
        "bdp": bdp,
        "wc": wc,
        "vw": vw,
        "ident": ident,
    }


_NC_CACHE = None
last_perf = {}


def kernel(s_t_hat, encoder_outputs, encoder_feature, enc_padding_mask, coverage,
           W_dp, b_dp, W_c, v_w, v_b):
    global _NC_CACHE
    if _NC_CACHE is None:
        _NC_CACHE = _build_nc()
    nc = _NC_CACHE

    s_t_hat = np.asarray(s_t_hat, dtype=np.float32)
    encoder_outputs = np.ascontiguousarray(encoder_outputs, dtype=np.float32)
    encoder_feature = np.ascontiguousarray(encoder_feature, dtype=np.float32)
    enc_padding_mask = np.ascontiguousarray(enc_padding_mask, dtype=np.float32)
    coverage = np.asarray(coverage, dtype=np.float32)
    # W_dp.T laid out partition-major: wTp[p, kc*H2+j] = W_dp[j, kc*128+p]
    wT = np.ascontiguousarray(
        np.asarray(W_dp, dtype=np.float32).T
        .reshape(KC, P, H2).transpose(1, 0, 2).reshape(P, KC * H2))
    bdp = np.asarray(b_dp, dtype=np.float32).reshape(1, H2)
    wc = np.asarray(W_c, dtype=np.float32).reshape(1, H2)
    vw = np.asarray(v_w, dtype=np.float32).reshape(1, H2)
    ident = np.eye(P, dtype=np.float32)

    in_maps = [
        _make_in_map(i, s_t_hat, encoder_outputs, encoder_feature,
                     enc_padding_mask, coverage, wT, bdp, wc, vw, ident)
        for i in range(NCORES)
    ]

    trace = os.environ.get("TRN_KERNEL_TRACE", "0") == "1"
    res = run_bass_kernel_spmd(nc, in_maps, core_ids=list(range(NCORES)),
                               trace=trace)
    last_perf["exec_time_ns"] = res.exec_time_ns
    last_perf["profile_json"] = res.profile_json
    outs = res.results

    c_t = np.concatenate([outs[i]["c_t"] for i in range(NCORES)], axis=0)
    attn = np.concatenate(
        [_unperm_bt(outs[i]["attn_p"]) for i in range(NCORES)], axis=0)
    cvn = np.concatenate(
        [_unperm_bt(outs[i]["cvn_p"]) for i in range(NCORES)], axis=0)
    return c_t, attn, cvn
